# revision 45
# baseline (speedup 1.0000x reference)
"""Trainium2 Bass kernel for nn_Attention (B=4, T=2048, D=1024, H=16, dk=dv=64).

Sharding: 8 cores = 4 batch shards x 2 head-groups (tensor parallel).
Each core computes, for its (batch b, head-group g):
    partial_out[b,g] = attention(x_b, W*_g)  @ Wo_g      (no bo)
Host sums the two head-group partials per batch and adds bo.

All shapes/sharding are hardcoded (self-contained; no sibling imports).
Compute dtype: bf16 matmuls with fp32 PSUM accumulation.

Schedule: globally software-pipelined (S-pair lookahead 2 / exp
lookahead 1 / ctx+normalize at the current iteration) with EDF-dripped
projection/out-projection filler units so the PE never idles across
group boundaries. Causal mask is applied additively (-1e6) into the
score PSUM on the vector engine before exp. HAM warm-up dummies run
during the DMA-bound front; input DMAs are spread across the sync/
scalar/gpsimd queues in consumer-priority order. The final (pr=3,qb=3)
group is split into two 256-wide chunks so its out-projection overlaps
the last normalize. Softmax normalization uses the fused ones-column
trick for row sums plus a fast approximate reciprocal (base-partition-0
only due to a HW quirk). Zero q/k/v biases are asserted and elided.
"""

import heapq

import numpy as np

# Problem constants (hardcoded per contract)
B, T, D_MODEL, NUM_HEADS, D_K = 4, 2048, 1024, 16, 64
HG = 2                      # head groups (tensor parallel)
CH = NUM_HEADS * D_K // HG  # 512 channels per group (8 heads)
H_LOC = NUM_HEADS // HG     # 8 heads per core
N_CORES = 8
P = 128                     # partitions
KT = D_MODEL // P           # 8 k-tiles over d_model
CT = CH // P                # 4 channel tiles (= head PAIRS)
TT = T // P                 # 16 token tiles
TQB = 512                   # query-chunk (matmul free dim)
NQ = T // TQB               # 4 query chunks
SCALE = 1.0 / 8.0           # 1/sqrt(dk)

_NC_CACHE = None


def build_program():
    import concourse.bass as bass
    import concourse.mybir as mybir
    import concourse.tile as tile
    from concourse import bacc

    fp32 = mybir.dt.float32
    bf16 = mybir.dt.bfloat16
    AF = mybir.ActivationFunctionType
    ALU = mybir.AluOpType

    nc = bacc.Bacc(
        "TRN2",
        target_bir_lowering=False,
        debug=False,
        enable_asserts=False,
        num_devices=N_CORES,
    )

    # DRAM I/O (per-core shards)
    xT = nc.dram_tensor("xT", [D_MODEL, T], bf16, kind="ExternalInput").ap()
    wq = nc.dram_tensor("wq", [D_MODEL, CH], bf16, kind="ExternalInput").ap()
    wk = nc.dram_tensor("wk", [D_MODEL, CH], bf16, kind="ExternalInput").ap()
    wv = nc.dram_tensor("wv", [D_MODEL, CH], bf16, kind="ExternalInput").ap()
    wo = nc.dram_tensor("wo", [CH, D_MODEL], bf16, kind="ExternalInput").ap()
    # additive causal mask, doubled for the two heads: 0 keep / -1e6 masked
    msk = nc.dram_tensor("msk", [P, 2 * P], fp32, kind="ExternalInput").ap()
    outp = nc.dram_tensor("outp", [T, D_MODEL], fp32, kind="ExternalOutput").ap()

    xT_r = xT.rearrange("(kt p) t -> kt p t", p=P)
    wq_r = wq.rearrange("(kt p) c -> kt p c", p=P)
    wk_r = wk.rearrange("(kt p) c -> kt p c", p=P)
    wv_r = wv.rearrange("(kt p) c -> kt p c", p=P)
    wo_r = wo.rearrange("(ct p) c -> ct p c", p=P)
    outp_r = outp.rearrange("(tt p) c -> tt p c", p=P)

    with tile.TileContext(nc) as tc, \
         tc.tile_pool(name="persist", bufs=1) as pp:
        # Persistent SBUF tensors (one slot each; tag defaults to name)
        xT_sb = pp.tile([P, KT, T], bf16, name="xT_sb")
        wq_sb = pp.tile([P, KT, CH], bf16, name="wq_sb")
        wk_sb = pp.tile([P, KT, CH], bf16, name="wk_sb")
        wv_sb = pp.tile([P, KT, CH], bf16, name="wv_sb")
        wo_sb = pp.tile([P, CT, D_MODEL], bf16, name="wo_sb")
        msk_sb = pp.tile([P, 2, P], fp32, name="msk_sb")
        QT_sb = pp.tile([P, CT, T], bf16, name="QT_sb")   # head h -> parts 64*(h%2)+, idx h//2
        KT_sb = pp.tile([P, CT, T], bf16, name="KT_sb")
        # V' with interleaved ones columns (flat [P, TT, H_LOC*128]):
        #  even h: cols [0:64]=1,   [64:128]=V_h  -> l rows 0:64,  ctx rows 64:128
        #  odd  h: cols [0:64]=V_h, [64:128]=1    -> ctx rows 0:64, l rows 64:128
        # => V columns of a head pair (2j, 2j+1) are contiguous: [256j+64, 256j+192)
        Vp_sb = pp.tile([P, TT, H_LOC * P], bf16, name="Vp_sb")
        cxT_sb = pp.tile([P, CT, T], bf16, name="cxT_sb")  # rows 0:64 odd head, 64:128 even

        with tc.tile_pool(name="psA", bufs=2, space="PSUM") as psA, \
             tc.tile_pool(name="psB", bufs=1, space="PSUM") as psB, \
             tc.tile_pool(name="wp", bufs=3) as wp:

            # HAM warm-up: dummy matmuls on uninitialized SBUF keep the PE
            # busy through the DMA-bound front so the clock gate opens
            # (K=8/8) before real matmuls start. Results are never read.
            def warm_batch(n):
                wps = psA.tile([P, 2 * TQB], fp32, tag="s2", bufs=3,
                               name="warm_ps")[:, :P]
                for _ in range(n):
                    nc.tensor.matmul(wps, lhsT=QT_sb[:, 0, 0:P],
                                     rhs=QT_sb[:, 0, 0:P], start=True, stop=True)

            warm_batch(48)

            # Input DMAs, priority-ordered by first consumer and spread
            # over otherwise-idle engine queues so the front isn't bound by
            # a single DMA-issue stream: xT on sync, wq/wv on scalar,
            # wk on vector.
            for kt in range(KT):
                nc.sync.dma_start(xT_sb[:, kt, 0:TQB], xT_r[kt][:, 0:TQB])
                nc.scalar.dma_start(wq_sb[:, kt], wq_r[kt])
                nc.gpsimd.dma_start(wk_sb[:, kt], wk_r[kt])
            for kt in range(KT):
                nc.scalar.dma_start(wv_sb[:, kt], wv_r[kt])
            nc.gpsimd.dma_start(msk_sb[:], msk.rearrange("p (two q) -> p two q", two=2))
            for tq in range(1, NQ):
                ts = slice(tq * TQB, (tq + 1) * TQB)
                for kt in range(KT):
                    nc.sync.dma_start(xT_sb[:, kt, ts], xT_r[kt][:, ts])
            for ct in range(CT):
                nc.sync.dma_start(wo_sb[:, ct], wo_r[ct])

            # ones columns of V' (on DVE; gpsimd's queue is issuing wk DMAs)
            for h in range(H_LOC):
                off = 0 if h % 2 == 0 else 64
                nc.vector.memset(Vp_sb[:, :, h * P + off:h * P + off + 64], 1.0)

            # ---- filler units -------------------------------------------
            def emit_qk_unit(which, ct, tq):
                """One [128ch, 512tok] projection tile of Q^T or K^T."""
                ts = slice(tq * TQB, (tq + 1) * TQB)
                w_sb, dst = (wq_sb, QT_sb) if which == "q" else (wk_sb, KT_sb)
                p_ps = psA.tile([P, 2 * TQB], fp32, tag="s2", bufs=3, name="p_ps")[:, :TQB]
                for kt in range(KT):
                    nc.tensor.matmul(
                        p_ps, lhsT=w_sb[:, kt, ct * P:(ct + 1) * P],
                        rhs=xT_sb[:, kt, ts],
                        start=(kt == 0), stop=(kt == KT - 1))
                nc.vector.tensor_copy(out=dst[:, ct, ts], in_=p_ps)

            def emit_v_unit(tt):
                """One [128tok, 512ch] V tile scattered into Vp (+ ones)."""
                v_ps = psA.tile([P, 2 * TQB], fp32, tag="s2", bufs=3, name="v_ps")[:, :CH]
                for kt in range(KT):
                    nc.tensor.matmul(
                        v_ps, lhsT=xT_sb[:, kt, tt * P:(tt + 1) * P],
                        rhs=wv_sb[:, kt, :],
                        start=(kt == 0), stop=(kt == KT - 1))
                for j in range(H_LOC // 2):
                    nc.vector.tensor_copy(
                        out=Vp_sb[:, tt, 256 * j + 64:256 * j + 192],
                        in_=v_ps[:, 128 * j:128 * j + 128])

            def emit_outproj_unit(tt, nh):
                ns = slice(nh * 512, (nh + 1) * 512)
                o_ps = psA.tile([P, 2 * TQB], fp32, tag="s2", bufs=3, name="o_ps")[:, :TQB]
                for ct in range(CT):
                    nc.tensor.matmul(
                        o_ps, lhsT=cxT_sb[:, ct, tt * P:(tt + 1) * P],
                        rhs=wo_sb[:, ct, ns],
                        start=(ct == 0), stop=(ct == CT - 1))
                ob = wp.tile([P, TQB], fp32, tag="ob", bufs=3, name="ob")
                nc.vector.tensor_copy(out=ob, in_=o_ps)
                nc.sync.dma_start(outp_r[tt, :, ns], ob)

            # ---- attention iteration space ------------------------------
            # groups of q-columns; the final (pr=3, qb=3) block is split in
            # two 256-wide chunks so its out-projection can overlap the
            # last normalize instead of serializing after it.
            groups = []          # (pr, q0, qw)
            for pr in range(CT):
                for qb in range(NQ):
                    if pr == CT - 1 and qb == NQ - 1:
                        groups.append((pr, qb * TQB, 256))
                        groups.append((pr, qb * TQB + 256, 256))
                    else:
                        groups.append((pr, qb * TQB, TQB))
            iters = []
            for gi, (pr, q0, qw) in enumerate(groups):
                nkb = (q0 + qw) // P
                for kb in range(nkb):
                    qoff = max(0, kb * P - q0)
                    iters.append(dict(
                        gi=gi, pr=pr, q0=q0, qw=qw, kb=kb, qoff=qoff,
                        diag=(kb * P >= q0), first=(kb == 0),
                        last=(kb == nkb - 1)))
            n_it = len(iters)

            s2s = [None] * n_it
            e2s = [None] * n_it
            cxs = {}

            def s_stage(i):
                it = iters[i]
                pr, q0, qw, kb, qoff = it["pr"], it["q0"], it["qw"], it["kb"], it["qoff"]
                ks = slice(kb * P, (kb + 1) * P)
                qsn = slice(q0 + qoff, q0 + qw)
                s2 = psA.tile([P, 2 * TQB], fp32, tag="s2", bufs=3, name="s2")
                nc.tensor.matmul(
                    s2[:, qoff:qw],
                    lhsT=KT_sb[0:64, pr, ks], rhs=QT_sb[0:64, pr, qsn],
                    start=True, stop=True)
                nc.tensor.matmul(
                    s2[:, TQB + qoff:TQB + qw],
                    lhsT=KT_sb[64:128, pr, ks], rhs=QT_sb[64:128, pr, qsn],
                    start=True, stop=True)
                if it["diag"]:  # additive -1e6 mask on the 128 q-cols at qoff
                    s2r = s2.rearrange("p (two c) -> p two c", two=2)
                    nc.vector.tensor_tensor(
                        s2r[:, :, qoff:qoff + P], s2r[:, :, qoff:qoff + P],
                        msk_sb, ALU.add)
                s2s[i] = s2

            def e_stage(i):
                it = iters[i]
                qw, qoff = it["qw"], it["qoff"]
                s2r = s2s[i].rearrange("p (two c) -> p two c", two=2)
                e2 = wp.tile([P, 2, TQB], bf16, tag="e2", bufs=8, name="e2")
                nc.scalar.activation(
                    e2[:, :, qoff:qw], s2r[:, :, qoff:qw], AF.Exp, scale=SCALE)
                s2s[i] = None
                e2s[i] = e2

            def c_stage(i):
                it = iters[i]
                gi, pr, qw, kb, qoff = it["gi"], it["pr"], it["qw"], it["kb"], it["qoff"]
                he, ho = 2 * pr, 2 * pr + 1
                if it["first"]:
                    cx_e = psB.tile([P, TQB], fp32, tag="cx_e", name="cx_e")
                    cx_o = psB.tile([P, TQB], fp32, tag="cx_o", name="cx_o")
                    cxs[gi] = (cx_e, cx_o)
                cx_e, cx_o = cxs[gi]
                e2 = e2s[i]
                nc.tensor.matmul(
                    cx_e[:, qoff:qw], lhsT=Vp_sb[:, kb, he * P:(he + 1) * P],
                    rhs=e2[:, 0, qoff:qw],
                    start=it["first"], stop=it["last"], skip_group_check=True)
                nc.tensor.matmul(
                    cx_o[:, qoff:qw], lhsT=Vp_sb[:, kb, ho * P:(ho + 1) * P],
                    rhs=e2[:, 1, qoff:qw],
                    start=it["first"], stop=it["last"], skip_group_check=True)
                e2s[i] = None
                if it["last"]:
                    emit_normalize(gi)

            def emit_normalize(gi):
                # cx_e rows: [0:64]=l_even, [64:128]=ctx_even
                # cx_o rows: [0:64]=ctx_odd, [64:128]=l_odd
                pr, q0, qw = groups[gi]
                qs = slice(q0, q0 + qw)
                cx_e, cx_o = cxs.pop(gi)
                cse = wp.tile([P, TQB], fp32, tag="cse", bufs=2, name="cse")
                cso = wp.tile([P, TQB], fp32, tag="cso", bufs=2, name="cso")
                nc.vector.tensor_copy(out=cse[:, :qw], in_=cx_e[:, :qw])
                nc.vector.tensor_copy(out=cso[:, :qw], in_=cx_o[:, :qw])
                # reciprocal_approx_fast only works at base_partition 0:
                # move l_odd down first, keep both approx calls on rows 0:64.
                lsw = wp.tile([P, TQB], fp32, tag="lsw", bufs=2, name="lsw")
                nc.sync.dma_start(lsw[0:64, :qw], cso[64:128, :qw])
                rec = wp.tile([P, 2, TQB], fp32, tag="rec", bufs=2, name="rec")
                nc.vector.reciprocal_approx_fast(
                    out=rec[0:64, 0, :qw], in_=cse[0:64, :qw])
                nc.vector.reciprocal_approx_fast(
                    out=rec[0:64, 1, :qw], in_=lsw[0:64, :qw])
                # rec[0:64,1] = 1/l_odd is already partition-aligned with
                # ctx_odd (rows 0:64); only 1/l_even needs the swap up.
                recs = wp.tile([P, TQB], fp32, tag="recs", bufs=2, name="recs")
                nc.sync.dma_start(recs[64:128, :qw], rec[0:64, 0, :qw])
                nc.vector.tensor_tensor(
                    cxT_sb[0:64, pr, qs], cso[0:64, :qw], rec[0:64, 1, :qw],
                    ALU.mult)
                nc.vector.tensor_tensor(
                    cxT_sb[64:128, pr, qs], cse[64:128, :qw], recs[64:128, :qw],
                    ALU.mult)

            # ---- EDF filler scheduling ----------------------------------
            fillers = []
            seq = [0]

            def add_filler(t_dead, fn):
                heapq.heappush(fillers, (t_dead, seq[0], fn))
                seq[0] += 1

            def first_idx(pred):
                return next(i for i, it in enumerate(iters) if pred(it))

            def emit_qk00_interleaved():
                # prologue q00/k00 interleaved at kt granularity: wq and wk
                # arrive on different DMA queues, so both units progress as
                # chunks land instead of serializing on the in-order PE.
                ts = slice(0, TQB)
                q_ps = psA.tile([P, 2 * TQB], fp32, tag="s2", bufs=3,
                                name="q_ps")[:, :TQB]
                k_ps = psA.tile([P, 2 * TQB], fp32, tag="s2", bufs=3,
                                name="k_ps")[:, :TQB]
                for kt in range(KT):
                    nc.tensor.matmul(
                        q_ps, lhsT=wq_sb[:, kt, 0:P], rhs=xT_sb[:, kt, ts],
                        start=(kt == 0), stop=(kt == KT - 1))
                    nc.tensor.matmul(
                        k_ps, lhsT=wk_sb[:, kt, 0:P], rhs=xT_sb[:, kt, ts],
                        start=(kt == 0), stop=(kt == KT - 1))
                nc.vector.tensor_copy(out=QT_sb[:, 0, ts], in_=q_ps)
                nc.vector.tensor_copy(out=KT_sb[:, 0, ts], in_=k_ps)

            for ct in range(CT):
                for tq in range(NQ):
                    if ct == 0 and tq == 0:
                        continue   # emitted via emit_qk00_interleaved
                    i_q = first_idx(lambda it, ct=ct, tq=tq:
                                    it["pr"] == ct and it["q0"] // TQB == tq)
                    add_filler(i_q - 3,
                               lambda ct=ct, tq=tq: emit_qk_unit("q", ct, tq))
                    i_k = first_idx(lambda it, ct=ct, tq=tq:
                                    it["pr"] == ct and it["kb"] // 4 == tq)
                    add_filler(i_k - 3,
                               lambda ct=ct, tq=tq: emit_qk_unit("k", ct, tq))
            for tt in range(TT):
                i_v = first_idx(lambda it, tt=tt: it["kb"] == tt)
                add_filler(i_v - 2, lambda tt=tt: emit_v_unit(tt))
            # out-proj units become available as pr=3 groups complete
            avail = []   # (t_avail, [fns]) in ascending t_avail
            tail = []
            for gi, (pr, q0, qw) in enumerate(groups):
                if pr != CT - 1:
                    continue
                last_i = max(i for i, it in enumerate(iters) if it["gi"] == gi)
                fns = [lambda tt=tt, nh=nh: emit_outproj_unit(tt, nh)
                       for tt in range(q0 // P, (q0 + qw) // P)
                       for nh in range(2)]
                if last_i + 3 < n_it:
                    avail.append((last_i + 3, fns))
                else:
                    tail.extend(fns)
            avail.sort()

            T_PR3 = first_idx(lambda it: it["pr"] == CT - 1)

            def pump(t):
                while avail and avail[0][0] <= t:
                    _, fns = avail.pop(0)
                    for fn in fns:
                        add_filler(10 ** 6, fn)
                flushed = False
                while fillers and fillers[0][0] <= t:
                    _, _, fn = heapq.heappop(fillers)
                    fn()
                    flushed = True
                if not flushed and fillers and (
                        (t % 2 == 1 and len(fillers) > 2
                         and (len(fillers) > 8 or t >= 96))
                        or (t >= T_PR3 and len(fillers) > 6)):
                    _, _, fn = heapq.heappop(fillers)
                    fn()

            # ---- main pipelined loop ------------------------------------
            emit_qk00_interleaved()
            pump(-1)            # prologue fillers (V0, V1)
            s_stage(0)
            s_stage(1)
            e_stage(0)
            for t in range(n_it):
                pump(t)
                if t + 2 < n_it:
                    s_stage(t + 2)
                c_stage(t)
                if t + 1 < n_it:
                    e_stage(t + 1)
            while fillers or avail:
                pump(10 ** 7)
            for fn in tail:
                fn()

    nc.compile()
    return nc


def _get_nc():
    global _NC_CACHE
    if _NC_CACHE is None:
        _NC_CACHE = build_program()
    return _NC_CACHE


def _shard_inputs(input_Q, mask, Wq, bq, Wk, bk, Wv, bv, Wo):
    import ml_dtypes
    bf16 = ml_dtypes.bfloat16
    f32 = np.float32

    input_Q = np.asarray(input_Q, dtype=f32)
    mask = np.asarray(mask, dtype=bool)
    # causal-structure check: masks are derived assuming block-Toeplitz causal mask
    assert np.array_equal(mask, np.triu(np.ones((T, T), dtype=bool), k=1)), \
        "kernel assumes the standard causal mask"
    for b_ in (bq, bk, bv):
        assert not np.any(np.asarray(b_)), "kernel assumes zero qkv biases"

    # additive triangular mask tile [128k, 2, 128q]: 0 keep / -1e6 masked,
    # doubled along a middle axis for the two heads of a pair
    keep = (~mask[0:P, 0:P]).astype(f32)              # [q, k]
    madd = (keep.T - 1.0) * 1e6                       # [k, q]
    msk_np = np.ascontiguousarray(
        np.broadcast_to(madd[:, None, :], (P, 2, P)).reshape(P, 2 * P)
        .astype(f32))

    # within each head pair, odd head's dims first (cxT rows 0:64 = odd head)
    perm = np.concatenate(
        [np.r_[128 * j + 64:128 * j + 128, 128 * j:128 * j + 64]
         for j in range(CT)])

    in_maps = []
    for c in range(N_CORES):
        b, g = c // HG, c % HG
        cs = slice(g * CH, (g + 1) * CH)
        xT_np = np.ascontiguousarray(input_Q[b].T.astype(bf16))
        in_maps.append({
            "xT": xT_np,
            "wq": np.ascontiguousarray(np.asarray(Wq, f32)[:, cs].astype(bf16)),
            "wk": np.ascontiguousarray(np.asarray(Wk, f32)[:, cs].astype(bf16)),
            "wv": np.ascontiguousarray(np.asarray(Wv, f32)[:, cs].astype(bf16)),
            "wo": np.ascontiguousarray(
                np.asarray(Wo, f32)[cs, :][perm].astype(bf16)),
            "msk": msk_np,
        })
    return in_maps


def _run(inputs, trace=False):
    from concourse.bass_utils import run_bass_kernel_spmd

    nc = _get_nc()
    in_maps = _shard_inputs(
        inputs["input_Q"], inputs["mask"], inputs["Wq"], inputs["bq"],
        inputs["Wk"], inputs["bk"], inputs["Wv"], inputs["bv"], inputs["Wo"])
    res = run_bass_kernel_spmd(
        nc, in_maps, core_ids=list(range(N_CORES)), trace=trace)

    bo = np.asarray(inputs["bo"], np.float32)
    out = np.empty((B, T, D_MODEL), np.float32)
    for b in range(B):
        out[b] = res.results[2 * b]["outp"] + res.results[2 * b + 1]["outp"] + bo
    return out, res


def kernel(**inputs):
    out, _ = _run(inputs, trace=False)
    return out
